# revision 7
# baseline (speedup 1.0000x reference)
"""Trainium2 Bass kernel: batched CRF forward algorithm (log partition).

Math (see reference): per sequence, forward scan over T=512 steps with
K=5 tags. transitions[START,:] = transitions[:,STOP] = -1e4, so in
exp-space the START row / STOP column of exp(transitions) are exact f32
zeros and only tags {0,1,2} carry state: K_eff = 3.

Exp-space recurrence per sequence (n, p in 0..2):
    a_1[n]   = exp(feat_0[n] + trans[n, START] - cbar)
    a_{t+1}[n] = sum_p W_t[n,p] * a_t[p],  W_t[n,p] = exp(feat_t[n] + trans[n,p] - cbar)
    alpha    = ln(sum_n exp(trans[STOP,n]) * a_T[n]) + sum(renorm logs) + T*cbar

cbar is a constant per-step log-growth estimate (host-derived from the
inputs); a periodic renormalization (every 32 steps) by the per-group
max keeps a in f32 range, with the logs of the maxes accumulated at the
end.

Distribution: pure data-parallel over the batch. Core c takes sequences
[c*1024, (c+1)*1024); on-chip layout is partition rho (128) x group g
(8) with seq = c*1024 + rho*8 + g. No collectives.

Engines: ScalarE (ACT) builds W = exp(feat + bias) chunks; VectorE runs
the sequential scan (broadcast-mul + segmented reduce per step); DMA
streams feats in t-chunks, double-buffered.
"""
import numpy as np

import concourse.bass as bass
import concourse.bacc as bacc
import concourse.tile as tile
from concourse import mybir
from concourse.bass_utils import run_bass_kernel_spmd

F32 = mybir.dt.float32
EXP = mybir.ActivationFunctionType.Exp
LN = mybir.ActivationFunctionType.Ln
MUL = mybir.AluOpType.mult
ADD = mybir.AluOpType.add
MAX = mybir.AluOpType.max
AXX = mybir.AxisListType.X

P = 128          # partitions
NT = 3           # effective tags {0,1,2}
K = 5            # raw tags per timestep
NCORES = 8
START = 3
STOP = 4


def build_program(T=512, G=8, TC=128, RN=32):
    """Build the per-core Bass program (identical on all 8 cores).

    T: sequence length; G: batch groups per partition (B_core = 128*G);
    TC: timestep chunk size; RN: renorm cadence in steps.
    """
    NCH = T // TC
    n_renorm = max(0, (T - 2 - RN) // RN + 1) if T - 1 >= RN else 0
    # renorms happen after steps t = RN, 2RN, ... while t <= T-32 guard below
    renorm_ts = [t for t in range(RN, T - 31, RN)]
    n_renorm = len(renorm_ts)

    nc = bacc.Bacc(
        "TRN2",
        target_bir_lowering=False,
        debug=False,
        enable_asserts=False,
        num_devices=NCORES,
    )
    feats = nc.dram_tensor("feats", [P * G, T * K], F32, kind="ExternalInput")
    aux = nc.dram_tensor("aux", [P, 16], F32, kind="ExternalInput")
    alpha = nc.dram_tensor("alpha", [P, G], F32, kind="ExternalOutput")

    fv = feats.ap().rearrange("(r g) (t k) -> r g t k", g=G, k=K)

    with tile.TileContext(nc) as tc:
        with (
            tc.tile_pool(name="auxp", bufs=1) as auxp,
            tc.tile_pool(name="rawp", bufs=2) as rawp,
            tc.tile_pool(name="st", bufs=1) as st,
        ):
            auxt = auxp.tile([P, 16], F32)
            nc.gpsimd.dma_start(out=auxt[:], in_=aux.ap())

            # Each instruction may carry at most ONE semaphore wait in this
            # walrus version. These absorber ops make each compute engine
            # observe the aux DMA early, so later ops never need a second
            # wait for it.
            act_scr = st.tile([P, 1], F32)
            dve_scr = st.tile([P, 1], F32)
            nc.scalar.copy(act_scr[:], auxt[:, 0:1])
            nc.vector.tensor_copy(dve_scr[:], auxt[:, 0:1])

            # W for the whole sequence stays resident in SBUF (147KB/part at
            # T=512); avoids pool slot-reuse waits on the ACT W-build ops.
            w_full = st.tile([P, G, T * 9], F32)
            w4 = w_full[:].rearrange("p g (t m) -> p g t m", m=9)

            a = st.tile([P, G * NT], F32)
            q = st.tile([P, G * NT * NT], F32)
            mbuf = st.tile([P, max(n_renorm, 1) * G], F32)
            rinv = st.tile([P, G], F32)

            a3 = a[:].rearrange("p (g w) -> p g w", w=NT)
            q4 = q[:].rearrange("p (g n z) -> p g n z", n=NT, z=NT)
            a4 = a3.unsqueeze(2).broadcast_to((P, G, NT, NT))

            r_i = 0
            for ch in range(NCH):
                raw = rawp.tile([P, G, TC * K], F32)
                raw4 = raw[:].rearrange("p g (t k) -> p g t k", k=K)
                nc.gpsimd.dma_start(
                    out=raw4, in_=fv[:, :, ch * TC : (ch + 1) * TC, :]
                )
                wc4 = w4[:, :, ch * TC : (ch + 1) * TC, :]
                for n in range(NT):
                    rin = raw4[:, :, :, n]
                    for pp in range(NT):
                        j = 3 * n + pp
                        nc.scalar.activation(
                            wc4[:, :, :, j], rin, EXP, bias=auxt[:, j : j + 1]
                        )
                if ch == 0:
                    # step 0: a_1[n] = exp(feat_0[n] + trans[n,START] - cbar)
                    for n in range(NT):
                        nc.scalar.activation(
                            a3[:, :, n],
                            raw4[:, :, 0, n],
                            EXP,
                            bias=auxt[:, 9 + n : 10 + n],
                        )
                t_lo = 1 if ch == 0 else 0
                for tl in range(t_lo, TC):
                    t = ch * TC + tl
                    wt = w4[:, :, t, :].rearrange("p g (n z) -> p g n z", z=NT)
                    nc.vector.tensor_tensor(q4, a4, wt, MUL)
                    nc.vector.tensor_reduce(a3, q4, axis=AXX, op=ADD)
                    if t in renorm_ts:
                        ms = mbuf[:, r_i * G : (r_i + 1) * G]
                        nc.vector.tensor_reduce(ms, a3, axis=AXX, op=MAX)
                        nc.vector.reciprocal(rinv[:], ms)
                        rb = rinv[:].unsqueeze(2).broadcast_to((P, G, NT))
                        nc.vector.tensor_tensor(a3, a3, rb, MUL)
                        r_i += 1
            assert r_i == n_renorm

            # terminal: s[g] = sum_n u[n] * a[g,n];  alpha = ln(s) + sum ln(m) + T*cbar
            ub = auxt[:, 12:15].unsqueeze(1).broadcast_to((P, G, NT))
            q3 = q[:, : G * NT].rearrange("p (g w) -> p g w", w=NT)
            nc.vector.tensor_tensor(q3, a3, ub, MUL)
            s8 = st.tile([P, G], F32)
            nc.vector.tensor_reduce(s8[:], q3, axis=AXX, op=ADD)
            sl = st.tile([P, G], F32)
            nc.scalar.activation(sl[:], s8[:], LN)
            at = st.tile([P, G], F32)
            if n_renorm > 0:
                mlog = st.tile([P, n_renorm * G], F32)
                nc.scalar.activation(mlog[:], mbuf[:, : n_renorm * G], LN)
                msum = st.tile([P, G], F32)
                nc.vector.tensor_reduce(
                    msum[:],
                    mlog[:].rearrange("p (r g) -> p g r", g=G),
                    axis=AXX,
                    op=ADD,
                )
                nc.vector.scalar_tensor_tensor(
                    at[:], sl[:], auxt[:, 15:16], msum[:], op0=ADD, op1=ADD
                )
            else:
                nc.vector.tensor_scalar_add(at[:], sl[:], auxt[:, 15:16])
            nc.gpsimd.dma_start(out=alpha.ap(), in_=at[:])
    nc.compile()
    return nc


def make_aux(transitions, cbar, T):
    tr = np.asarray(transitions, np.float32)
    row = np.zeros(16, np.float32)
    row[0:9] = (tr[:NT, :NT] - cbar).reshape(9)
    row[9:12] = tr[:NT, START] - cbar
    row[12:15] = np.exp(tr[STOP, :NT])
    row[15] = T * cbar
    return np.ascontiguousarray(np.broadcast_to(row, (P, 16)))


def compute_cbar(feats, transitions):
    tr = np.asarray(transitions, np.float64)
    m = np.exp(tr[:NT, :NT])
    cbar = float(np.log(m.sum(1)).mean())
    cbar += float(np.asarray(feats[::257, :, :NT], np.float64).max(axis=-1).mean())
    return cbar


_prog = None


def kernel(feats, transitions):
    global _prog
    feats = np.ascontiguousarray(np.asarray(feats, np.float32))
    B, T, Kk = feats.shape
    assert (B, T, Kk) == (8192, 512, 5)
    if _prog is None:
        _prog = build_program(T=T)
    cbar = compute_cbar(feats, transitions)
    aux = make_aux(transitions, cbar, T)
    bc = B // NCORES
    fr = feats.reshape(NCORES, bc, T * Kk)
    in_maps = [{"feats": fr[c], "aux": aux} for c in range(NCORES)]
    res = run_bass_kernel_spmd(_prog, in_maps, core_ids=list(range(NCORES))).results
    out = np.concatenate(
        [np.asarray(res[c]["alpha"], np.float32).reshape(bc) for c in range(NCORES)]
    )
    return out
